# revision 21
# baseline (speedup 1.0000x reference)
"""Trainium2 Bass kernel for nn_Block_927712936453 (dense transformer block,
cross-attention x/y, DIM=384, HEADS=6, HIDDEN=1536).

Sharding: 8 cores = 4 batches x 2 token-halves, fully data-parallel (no
collectives). Each core receives batch-b x/y with its own token half ordered
first (softmax over keys is permutation invariant), computes LN1+QKV over all
2304 tokens (K/V are needed in full), then attention + proj + MLP only for its
own 1152 query tokens. The host reassembles the halves.

LayerNorm affine params are folded into the downstream matmul weights on the
host (exact): qkv_w' = qkv_w * n1w, qkv_b' = qkv_w @ n1b; fc1 likewise with
norm2. On-chip LN is just stats + (x-mean)*rstd.

Attention uses the S^T formulation (keys on partitions): S^T = K_h^T.T @ Q_h^T,
P = exp(S*scale) -> PV accumulates o^T feature-major. V is augmented with 64
all-ones columns so PSUM rows 64:128 hold the softmax denominators replicated;
a single approx-reciprocal + multiply normalizes.
"""
import sys

if "/opt/trn_rl_repo" not in sys.path:
    sys.path.insert(0, "/opt/trn_rl_repo")

from contextlib import ExitStack

import ml_dtypes
import numpy as np

import concourse.bass as bass
import concourse.tile as tile
from concourse import bacc, mybir
from concourse.bass import ds, ts
from concourse.bass_utils import run_bass_kernel_spmd
from concourse.masks import make_identity

F32 = mybir.dt.float32
BF16 = mybir.dt.bfloat16
AF = mybir.ActivationFunctionType
OP = mybir.AluOpType

B, NX, NY, C = 4, 2048, 256, 384
H, D = 6, 64
HID = 1536
SCALE = D ** -0.5
EPS = 1e-5
N_CORES = 8

TOWN_X = NX // 2                       # 1024 own x tokens per core
TOWN_Y = NY // 2                       # 128 own y tokens
TOWN = TOWN_X + TOWN_Y                 # 1152 own tokens
TFULL = NX + NY                        # 2304 total tokens
P = 128
NT_FULL = TFULL // P                   # 18 token tiles
NT_OWN = TOWN // P                     # 9 own token tiles
NKT_X = NX // P                        # 16 key tiles for x-attention
NKT_ALL = NT_FULL                      # 18 key tiles for y-attention
NC_C = C // P                          # 3 feature tiles
NH_HID = HID // P                      # 12 hidden tiles


def _chunks(total, step=512):
    out, o = [], 0
    while o < total:
        w = min(step, total - o)
        out.append((o, w))
        o += w
    return out


def _bcast_ap(ap, parts=P):
    """Broadcast a 1-D DRAM tensor across `parts` partitions (DMA-side)."""
    return bass.AP(tensor=ap.tensor, offset=ap.offset, ap=[[0, parts], *ap.ap])


def build_nc(debug_dump=False):
    nc = bacc.Bacc("TRN2", target_bir_lowering=False, debug=False)

    xb = nc.dram_tensor("xb", [NX, C], F32, kind="ExternalInput").ap()
    yb = nc.dram_tensor("yb", [NY, C], F32, kind="ExternalInput").ap()
    qkvwT = nc.dram_tensor("qkvwT", [C, 3 * C], BF16, kind="ExternalInput").ap()
    qkvb = nc.dram_tensor("qkvb", [3 * C], F32, kind="ExternalInput").ap()
    projwT = nc.dram_tensor("projwT", [C, C], BF16, kind="ExternalInput").ap()
    projb = nc.dram_tensor("projb", [C], F32, kind="ExternalInput").ap()
    fc1wT = nc.dram_tensor("fc1wT", [C, HID], BF16, kind="ExternalInput").ap()
    fc1b = nc.dram_tensor("fc1b", [HID], F32, kind="ExternalInput").ap()
    fc2wT = nc.dram_tensor("fc2wT", [HID, C], BF16, kind="ExternalInput").ap()
    fc2b = nc.dram_tensor("fc2b", [C], F32, kind="ExternalInput").ap()
    out_x = nc.dram_tensor("out_x", [TOWN_X, C], F32, kind="ExternalOutput").ap()
    out_y = nc.dram_tensor("out_y", [TOWN_Y, C], F32, kind="ExternalOutput").ap()

    def tok_rows(i):  # cat order: x_own | x_oth | y_own | y_oth
        if i < 16:
            return xb[i * P:(i + 1) * P, :]
        return yb[(i - 16) * P:(i - 15) * P, :]

    own_tiles = list(range(8)) + [16]

    with tile.TileContext(nc) as tc, ExitStack() as top:
        persist = top.enter_context(tc.tile_pool(name="persist", bufs=1))
        wpool = top.enter_context(tc.tile_pool(name="weights", bufs=1))

        # ---- persistent SBUF tensors ----
        catT = [persist.tile([P, TFULL], BF16, tag=f"catT{c}", name=f"catT{c}")
                for c in range(NC_C)]
        KT = [persist.tile([P, TFULL], BF16, tag=f"KT{c}", name=f"KT{c}")
              for c in range(NC_C)]
        QT = [persist.tile([P, TOWN], BF16, tag=f"QT{c}", name=f"QT{c}")
              for c in range(NC_C)]
        Vt = [persist.tile([P, H, P], BF16, tag=f"V{i}", name=f"V{i}")
              for i in range(NT_FULL)]
        oT = [persist.tile([P, TOWN], BF16, tag=f"oT{c}", name=f"oT{c}")
              for c in range(NC_C)]
        ln2T = [persist.tile([P, TOWN], BF16, tag=f"l2T{c}", name=f"l2T{c}")
                for c in range(NC_C)]
        hT = [persist.tile([P, TOWN], BF16, tag=f"hT{m}", name=f"hT{m}")
              for m in range(NH_HID)]
        res = [persist.tile([P, C], F32, tag=f"res{j}", name=f"res{j}")
               for j in range(NT_OWN)]

        # ---- weights / constants ----
        qkvw_sb = [wpool.tile([P, 3 * C], BF16, tag=f"qkvw{c}", name=f"qkvw{c}")
                   for c in range(NC_C)]
        projw_sb = [wpool.tile([P, C], BF16, tag=f"projw{c}", name=f"projw{c}")
                    for c in range(NC_C)]
        fc1w_sb = [wpool.tile([P, HID], BF16, tag=f"fc1w{c}", name=f"fc1w{c}")
                   for c in range(NC_C)]
        fc2w_sb = [wpool.tile([P, C], BF16, tag=f"fc2w{m}", name=f"fc2w{m}")
                   for m in range(NH_HID)]
        for c in range(NC_C):
            nc.gpsimd.dma_start(qkvw_sb[c][:], qkvwT[ts(c, P), :])
            nc.gpsimd.dma_start(projw_sb[c][:], projwT[ts(c, P), :])
            nc.gpsimd.dma_start(fc1w_sb[c][:], fc1wT[ts(c, P), :])
        for m in range(NH_HID):
            nc.gpsimd.dma_start(fc2w_sb[m][:], fc2wT[ts(m, P), :])

        projb_bc = wpool.tile([P, C], F32, tag="projb", name="projb")
        fc2b_bc = wpool.tile([P, C], F32, tag="fc2b", name="fc2b")
        vb_bc = wpool.tile([P, C], F32, tag="vb", name="vb")
        for dst, src in ((projb_bc, projb), (fc2b_bc, fc2b),
                         (vb_bc, qkvb[ds(2 * C, C)])):
            nc.gpsimd.dma_start(out=dst[:], in_=_bcast_ap(src))
        # feature-major biases: [128, tile-idx]
        qkvb_sb = wpool.tile([P, 9], F32, tag="qkvb", name="qkvb")
        nc.gpsimd.dma_start(qkvb_sb[:], qkvb.rearrange("(i p) -> p i", p=P))
        fc1b_sb = wpool.tile([P, NH_HID], F32, tag="fc1b", name="fc1b")
        nc.gpsimd.dma_start(fc1b_sb[:], fc1b.rearrange("(i p) -> p i", p=P))

        ident = wpool.tile([P, P], BF16, tag="ident", name="ident")
        make_identity(nc, ident[:])
        eps_sb = wpool.tile([P, 1], F32, tag="eps", name="eps")
        nc.vector.memset(eps_sb[:], EPS)

        # ones columns of the augmented V (softmax denominator replication);
        # ones FIRST so the denominators land at PSUM partition base 0
        for i in range(NT_FULL):
            nc.gpsimd.memset(Vt[i][:, :, 0:D], 1.0)

        def ln_apply(st_pool, xt, out_bf16):
            """(x - mean(x)) * rsqrt(var + eps); affine folded into weights."""
            stats = st_pool.tile([P, 6], F32, tag="stats", name="stats")
            nc.vector.bn_stats(out=stats[:], in_=xt[:])
            mv = st_pool.tile([P, 2], F32, tag="mv", name="mv")
            nc.vector.bn_aggr(out=mv[:], in_=stats[:])
            rstd = st_pool.tile([P, 1], F32, tag="rstd", name="rstd")
            nc.scalar.activation(out=rstd[:], in_=mv[:, 1:2], func=AF.Sqrt,
                                 bias=eps_sb[:], scale=1.0)
            nc.vector.reciprocal(out=rstd[:], in_=rstd[:])
            nc.vector.tensor_scalar(out=out_bf16[:], in0=xt[:], scalar1=mv[:, 0:1],
                                    scalar2=rstd[:], op0=OP.subtract, op1=OP.mult)


        def make_qkv_chunks(m, pool, tag="mm"):
            ks, qs = [], []
            for (o, w) in _chunks(TFULL):
                def gk(o=o, w=w, m=m, pool=pool, tag=tag):
                    ps = pool.tile([P, 512], F32, tag=tag, name="mm")
                    for c in range(NC_C):
                        nc.tensor.matmul(ps[:, :w],
                                         qkvw_sb[c][:, ds(C + m * P, P)],
                                         catT[c][:, ds(o, w)],
                                         start=(c == 0), stop=(c == NC_C - 1))
                    nc.vector.tensor_scalar_add(out=KT[m][:, ds(o, w)],
                                                in0=ps[:, :w],
                                                scalar1=qkvb_sb[:, 3 + m:4 + m])
                ks.append(gk)
            for (src_o, w, dst_o) in ((0, 512, 0), (512, 512, 512),
                                      (2048, P, 1024)):
                def gq(src_o=src_o, w=w, dst_o=dst_o, m=m, pool=pool, tag=tag):
                    ps = pool.tile([P, 512], F32, tag=tag, name="mm")
                    for c in range(NC_C):
                        nc.tensor.matmul(ps[:, :w],
                                         qkvw_sb[c][:, ds(m * P, P)],
                                         catT[c][:, ds(src_o, w)],
                                         start=(c == 0), stop=(c == NC_C - 1))
                    nc.vector.tensor_scalar_add(out=QT[m][:, ds(dst_o, w)],
                                                in0=ps[:, :w],
                                                scalar1=qkvb_sb[:, m:m + 1])
                qs.append(gq)
            return ks, qs

        def make_v(i, pool):
            def gv(i=i, pool=pool):
                ps = pool.tile([P, 512], F32, tag="mm", name="mm")
                for c in range(NC_C):
                    nc.tensor.matmul(ps[:, :C], catT[c][:, ts(i, P)],
                                     qkvw_sb[c][:, ds(2 * C, C)],
                                     start=(c == 0), stop=(c == NC_C - 1))
                nc.vector.tensor_add(Vt[i][:, :, D:P],
                                     ps[:, :C].rearrange("p (h d) -> p h d", h=H),
                                     vb_bc[:].rearrange("p (h d) -> p h d", h=H))
            return gv

        def make_proj(j, pool, tag="mm"):
            def gp(j=j, pool=pool, tag=tag):
                ps = pool.tile([P, 512], F32, tag=tag, name="mm")
                for c in range(NC_C):
                    nc.tensor.matmul(ps[:, :C], oT[c][:, ts(j, P)],
                                     projw_sb[c][:],
                                     start=(c == 0), stop=(c == NC_C - 1))
                nc.vector.tensor_add(res[j][:], ps[:, :C], res[j][:])
            return gp

        # ================= Phase A: LN1 + transpose to feature-major ========
        with ExitStack() as sa:
            xt_pool = sa.enter_context(tc.tile_pool(name="xt", bufs=3))
            st_pool = sa.enter_context(tc.tile_pool(name="lnstat", bufs=3))
            lt_pool = sa.enter_context(tc.tile_pool(name="lntok", bufs=3))
            tr_psum = sa.enter_context(tc.tile_pool(name="trps", bufs=3, space="PSUM"))
            mm_psum = sa.enter_context(tc.tile_pool(name="mmps", bufs=3, space="PSUM"))

            k0, q0 = make_qkv_chunks(0, mm_psum)
            after_tile = {3: [k0[0], q0[0]], 7: [k0[1], q0[1]], 11: [k0[2]],
                          15: [k0[3]], 16: [q0[2]], 17: [k0[4]]}
            for i in range(NT_FULL):
                xt = xt_pool.tile([P, C], F32, tag="xt", name="xt")
                nc.sync.dma_start(xt[:], tok_rows(i))
                if i in own_tiles:
                    nc.gpsimd.tensor_add(res[own_tiles.index(i)][:], xt[:],
                                         projb_bc[:])
                ln_tok = lt_pool.tile([P, C], BF16, tag="lntok", name="lntok")
                ln_apply(st_pool, xt, ln_tok)
                for c in range(NC_C):
                    tp = tr_psum.tile([P, P], BF16, tag="tr", name="tr")
                    nc.tensor.transpose(tp[:], ln_tok[:, ts(c, P)], ident[:])
                    nc.scalar.copy(catT[c][:, ts(i, P)], tp[:])
                make_v(i, mm_psum)()
                for g in after_tile.pop(i, []):
                    g()

        # ============ Phase B+C: QKV + attention, interleaved ===============
        # V and QKV(m=0) first; then attention(m) runs with QKV(m+1) matmul
        # groups sprinkled into its key-tile loop as TensorE filler while the
        # Scalar engine (exp) is the binding resource.
        with ExitStack() as sc:
            s_psum = sc.enter_context(tc.tile_pool(name="sps", bufs=3, space="PSUM"))
            o_psum = sc.enter_context(tc.tile_pool(name="ops", bufs=2, space="PSUM"))
            p_pool = sc.enter_context(tc.tile_pool(name="psb", bufs=5))
            r_pool = sc.enter_context(tc.tile_pool(name="recip", bufs=4))

            def norm_out(ops_t, m, po, qo, qw):
                rec = r_pool.tile([D, 512], F32, tag="rec", name="rec")
                nc.vector.reciprocal_approx_fast(out=rec[:, :qw],
                                                 in_=ops_t[0:D, :qw])
                nc.vector.tensor_mul(oT[m][po:po + D, ds(qo, qw)],
                                     ops_t[D:P, :qw], rec[:, :qw])

            for m in range(NC_C):
                if m + 1 < NC_C:
                    ks, qs = make_qkv_chunks(m + 1, s_psum, tag="s")
                    filler_ci = {0: ks[:2] + qs[:1], 1: ks[2:4] + qs[1:2]}
                    filler_y = ks[4:] + qs[2:]
                else:
                    filler_ci = {0: [],
                                 1: [make_proj(j, s_psum, tag="s") for j in range(4)]}
                    filler_y = [make_proj(j, s_psum, tag="s") for j in range(4, 8)]
                # ---- x-attention: 2 chunks of 512 queries ----
                for ci in range(2):
                    qo = ci * 512
                    opair = [o_psum.tile([P, 512], F32, tag="o", name="o")
                             for _ in range(2)]
                    for kt in range(NKT_X):
                        sps = s_psum.tile([P, 1024], F32, tag="s", name="s")
                        for par in range(2):
                            po = par * D
                            nc.tensor.matmul(sps[:, ds(par * 512, 512)],
                                             KT[m][po:po + D, ts(kt, P)],
                                             QT[m][po:po + D, ds(qo, 512)],
                                             tile_position=(po, 0))
                        pt = p_pool.tile([P, 1024], BF16, tag="p", name="p")
                        nc.scalar.activation(out=pt[:], in_=sps[:],
                                             func=AF.Exp, scale=SCALE)
                        for par in range(2):
                            nc.tensor.matmul(opair[par][:],
                                             Vt[kt][:, 2 * m + par, :],
                                             pt[:, ds(par * 512, 512)],
                                             start=(kt == 0), stop=(kt == NKT_X - 1))
                        if kt % 4 == 1 and filler_ci[ci]:
                            filler_ci[ci].pop(0)()
                    for par in range(2):
                        norm_out(opair[par], m, par * D, qo, 512)

                # ---- y-attention: 128 queries, 4 key tiles per S tile ----
                oy = [o_psum.tile([P, 512], F32, tag="o", name="o")
                      for _ in range(2)]
                groups = [list(range(g, min(g + 4, NKT_ALL)))
                          for g in range(0, NKT_ALL, 4)]
                for gkts in groups:
                    sps = s_psum.tile([P, 1024], F32, tag="s", name="s")
                    for gi, kt in enumerate(gkts):
                        for par in range(2):
                            po = par * D
                            nc.tensor.matmul(
                                sps[:, ds(par * 512 + gi * TOWN_Y, TOWN_Y)],
                                KT[m][po:po + D, ts(kt, P)],
                                QT[m][po:po + D, ds(TOWN_X, TOWN_Y)],
                                tile_position=(po, 0))
                    pt = p_pool.tile([P, 1024], BF16, tag="p", name="p")
                    nc.scalar.activation(out=pt[:], in_=sps[:],
                                         func=AF.Exp, scale=SCALE)
                    for gi, kt in enumerate(gkts):
                        for par in range(2):
                            nc.tensor.matmul(
                                oy[par][:, 0:TOWN_Y],
                                Vt[kt][:, 2 * m + par, :],
                                pt[:, ds(par * 512 + gi * TOWN_Y, TOWN_Y)],
                                start=(kt == 0), stop=(kt == NKT_ALL - 1))
                    if filler_y:
                        filler_y.pop(0)()
                for par in range(2):
                    norm_out(oy[par], m, par * D, TOWN_X, TOWN_Y)
                while filler_ci[0] or filler_ci[1] or filler_y:
                    (filler_ci[0] or filler_ci[1] or filler_y).pop(0)()
            make_proj(8, s_psum, tag="s")()

        # ============ Phase D/E/F: proj+residual, LN2, MLP ==================
        with ExitStack() as sd:
            mm_psum = sd.enter_context(tc.tile_pool(name="mmps2", bufs=2, space="PSUM"))
            tr_psum = sd.enter_context(tc.tile_pool(name="trps2", bufs=2, space="PSUM"))
            st_pool = sd.enter_context(tc.tile_pool(name="lnstat2", bufs=3))
            lt_pool = sd.enter_context(tc.tile_pool(name="lntok2", bufs=3))
            tmp_pool = sd.enter_context(tc.tile_pool(name="tmp", bufs=3))
            out_pool = sd.enter_context(tc.tile_pool(name="outp", bufs=3))

            for j in range(NT_OWN):
                ln_tok = lt_pool.tile([P, C], BF16, tag="lntok", name="lntok")
                ln_apply(st_pool, res[j], ln_tok)
                for c in range(NC_C):
                    tp = tr_psum.tile([P, P], BF16, tag="tr", name="tr")
                    nc.tensor.transpose(tp[:], ln_tok[:, ts(c, P)], ident[:])
                    nc.vector.tensor_copy(ln2T[c][:, ts(j, P)], tp[:])

            for m in range(NH_HID):
                for (o, w) in ((0, 1024), (1024, TOWN_Y)):
                    ps = mm_psum.tile([P, 1024], F32, tag="mmw", name="mmw", bufs=2)
                    for (f, fw) in _chunks(w):
                        for c in range(NC_C):
                            nc.tensor.matmul(ps[:, ds(f, fw)],
                                             fc1w_sb[c][:, ts(m, P)],
                                             ln2T[c][:, ds(o + f, fw)],
                                             start=(c == 0), stop=(c == NC_C - 1))
                    nc.scalar.activation(out=hT[m][:, ds(o, w)], in_=ps[:, :w],
                                         func=AF.Gelu, bias=fc1b_sb[:, m:m + 1],
                                         scale=1.0)

            for j in range(NT_OWN):
                ps = mm_psum.tile([P, 512], F32, tag="mm", name="mm")
                for m in range(NH_HID):
                    nc.tensor.matmul(ps[:, :C], hT[m][:, ts(j, P)], fc2w_sb[m][:],
                                     start=(m == 0), stop=(m == NH_HID - 1))
                t = tmp_pool.tile([P, C], F32, tag="mlp", name="mlp")
                nc.vector.tensor_add(t[:], ps[:, :C], fc2b_bc[:])
                ot = out_pool.tile([P, C], F32, tag="out", name="out")
                nc.vector.tensor_add(ot[:], t[:], res[j][:])
                if j < 8:
                    nc.sync.dma_start(out_x[ts(j, P), :], ot[:])
                else:
                    nc.sync.dma_start(out_y[:, :], ot[:])

            if debug_dump:
                for c in range(NC_C):
                    for nm, t in (("KT", KT), ("QT", QT), ("oT", oT),
                                  ("catT", catT)):
                        dt = nc.dram_tensor(f"d_{nm}{c}", list(t[c].shape), BF16,
                                            kind="ExternalOutput").ap()
                        nc.sync.dma_start(dt[:], t[c][:])
                for i in (0, 16):
                    dt = nc.dram_tensor(f"d_V{i}", list(Vt[i].shape), BF16,
                                        kind="ExternalOutput").ap()
                    nc.sync.dma_start(dt[:], Vt[i][:])

    nc.compile()
    return nc


_NC_CACHE = None


def _get_nc():
    global _NC_CACHE
    if _NC_CACHE is None:
        _NC_CACHE = build_nc()
    return _NC_CACHE


def make_in_maps(x, y, norm1_w, norm1_b, norm2_w, norm2_b, qkv_w,
                 proj_w, proj_b, fc1_w, fc1_b, fc2_w, fc2_b):
    bf = ml_dtypes.bfloat16
    f32 = np.float32
    qkv_w = np.asarray(qkv_w, f32)
    fc1_w = np.asarray(fc1_w, f32)
    n1w = np.asarray(norm1_w, f32)
    n1b = np.asarray(norm1_b, f32)
    n2w = np.asarray(norm2_w, f32)
    n2b = np.asarray(norm2_b, f32)
    qkv_w_eff = qkv_w * n1w[None, :]
    qkv_b_eff = qkv_w @ n1b
    fc1_w_eff = fc1_w * n2w[None, :]
    fc1_b_eff = np.asarray(fc1_b, f32) + fc1_w @ n2b
    shared = {
        "qkvwT": np.ascontiguousarray(qkv_w_eff.T.astype(bf)),
        "qkvb": np.ascontiguousarray(qkv_b_eff, f32),
        "projwT": np.ascontiguousarray(np.asarray(proj_w, f32).T.astype(bf)),
        "projb": np.ascontiguousarray(proj_b, f32),
        "fc1wT": np.ascontiguousarray(fc1_w_eff.T.astype(bf)),
        "fc1b": np.ascontiguousarray(fc1_b_eff, f32),
        "fc2wT": np.ascontiguousarray(np.asarray(fc2_w, f32).T.astype(bf)),
        "fc2b": np.ascontiguousarray(fc2_b, f32),
    }
    x = np.asarray(x, f32)
    y = np.asarray(y, f32)
    in_maps = []
    for core in range(N_CORES):
        b, s = core // 2, core % 2
        if s == 0:
            xbp, ybp = x[b], y[b]
        else:
            xbp = np.concatenate([x[b, TOWN_X:], x[b, :TOWN_X]], axis=0)
            ybp = np.concatenate([y[b, TOWN_Y:], y[b, :TOWN_Y]], axis=0)
        in_maps.append({"xb": np.ascontiguousarray(xbp),
                        "yb": np.ascontiguousarray(ybp), **shared})
    return in_maps


def assemble(results):
    new_x = np.empty((B, NX, C), np.float32)
    new_y = np.empty((B, NY, C), np.float32)
    for core in range(N_CORES):
        b, s = core // 2, core % 2
        new_x[b, s * TOWN_X:(s + 1) * TOWN_X] = results[core]["out_x"]
        new_y[b, s * TOWN_Y:(s + 1) * TOWN_Y] = results[core]["out_y"]
    return new_x, new_y


def run(in_maps, trace=False):
    return run_bass_kernel_spmd(_get_nc(), in_maps,
                                core_ids=list(range(N_CORES)), trace=trace)


def kernel(**inputs):
    in_maps = make_in_maps(**inputs)
    res = run(in_maps, trace=False)
    return assemble(res.results)


# revision 23
# speedup vs baseline: 1.0559x; 1.0559x over previous
"""Trainium2 Bass kernel for nn_Block_927712936453 (dense transformer block,
cross-attention x/y, DIM=384, HEADS=6, HIDDEN=1536).

Sharding: 8 cores = 4 batches x 2 token-halves, fully data-parallel (no
collectives). Each core receives batch-b x/y with its own token half ordered
first (softmax over keys is permutation invariant), computes LN1+QKV over all
2304 tokens (K/V are needed in full), then attention + proj + MLP only for its
own 1152 query tokens. The host reassembles the halves.

LayerNorm affine params are folded into the downstream matmul weights on the
host (exact): qkv_w' = qkv_w * n1w, qkv_b' = qkv_w @ n1b; fc1 likewise with
norm2. On-chip LN is just stats + (x-mean)*rstd.

Attention uses the S^T formulation (keys on partitions): S^T = K_h^T.T @ Q_h^T,
P = exp(S*scale) -> PV accumulates o^T feature-major. V is augmented with 64
all-ones columns so PSUM rows 64:128 hold the softmax denominators replicated;
a single approx-reciprocal + multiply normalizes.
"""
import sys

if "/opt/trn_rl_repo" not in sys.path:
    sys.path.insert(0, "/opt/trn_rl_repo")

from contextlib import ExitStack

import ml_dtypes
import numpy as np

import concourse.bass as bass
import concourse.tile as tile
from concourse import bacc, mybir
from concourse.bass import ds, ts
from concourse.bass_utils import run_bass_kernel_spmd
from concourse.masks import make_identity

F32 = mybir.dt.float32
BF16 = mybir.dt.bfloat16
AF = mybir.ActivationFunctionType
OP = mybir.AluOpType

B, NX, NY, C = 4, 2048, 256, 384
H, D = 6, 64
HID = 1536
SCALE = D ** -0.5
EPS = 1e-5
N_CORES = 8

TOWN_X = NX // 2                       # 1024 own x tokens per core
TOWN_Y = NY // 2                       # 128 own y tokens
TOWN = TOWN_X + TOWN_Y                 # 1152 own tokens
TFULL = NX + NY                        # 2304 total tokens
P = 128
NT_FULL = TFULL // P                   # 18 token tiles
NT_OWN = TOWN // P                     # 9 own token tiles
NKT_X = NX // P                        # 16 key tiles for x-attention
NKT_ALL = NT_FULL                      # 18 key tiles for y-attention
NC_C = C // P                          # 3 feature tiles
NH_HID = HID // P                      # 12 hidden tiles


def _chunks(total, step=512):
    out, o = [], 0
    while o < total:
        w = min(step, total - o)
        out.append((o, w))
        o += w
    return out


def _bcast_ap(ap, parts=P):
    """Broadcast a 1-D DRAM tensor across `parts` partitions (DMA-side)."""
    return bass.AP(tensor=ap.tensor, offset=ap.offset, ap=[[0, parts], *ap.ap])


def build_nc(debug_dump=False):
    nc = bacc.Bacc("TRN2", target_bir_lowering=False, debug=False)

    xb = nc.dram_tensor("xb", [NX, C], F32, kind="ExternalInput").ap()
    yb = nc.dram_tensor("yb", [NY, C], F32, kind="ExternalInput").ap()
    qkvwT = nc.dram_tensor("qkvwT", [C, 3 * C], BF16, kind="ExternalInput").ap()
    qkvb = nc.dram_tensor("qkvb", [3 * C], F32, kind="ExternalInput").ap()
    projwT = nc.dram_tensor("projwT", [C, C], BF16, kind="ExternalInput").ap()
    projb = nc.dram_tensor("projb", [C], F32, kind="ExternalInput").ap()
    fc1wT = nc.dram_tensor("fc1wT", [C, HID], BF16, kind="ExternalInput").ap()
    fc1b = nc.dram_tensor("fc1b", [HID], F32, kind="ExternalInput").ap()
    fc2wT = nc.dram_tensor("fc2wT", [HID, C], BF16, kind="ExternalInput").ap()
    fc2b = nc.dram_tensor("fc2b", [C], F32, kind="ExternalInput").ap()
    out_x = nc.dram_tensor("out_x", [TOWN_X, C], F32, kind="ExternalOutput").ap()
    out_y = nc.dram_tensor("out_y", [TOWN_Y, C], F32, kind="ExternalOutput").ap()

    def tok_rows(i):  # cat order: x_own | x_oth | y_own | y_oth
        if i < 16:
            return xb[i * P:(i + 1) * P, :]
        return yb[(i - 16) * P:(i - 15) * P, :]

    own_tiles = list(range(8)) + [16]

    with tile.TileContext(nc) as tc, ExitStack() as top:
        persist = top.enter_context(tc.tile_pool(name="persist", bufs=1))
        wpool = top.enter_context(tc.tile_pool(name="weights", bufs=1))

        # ---- persistent SBUF tensors ----
        catT = [persist.tile([P, TFULL], BF16, tag=f"catT{c}", name=f"catT{c}")
                for c in range(NC_C)]
        KT = [persist.tile([P, TFULL], BF16, tag=f"KT{c}", name=f"KT{c}")
              for c in range(NC_C)]
        QT = [persist.tile([P, TOWN], BF16, tag=f"QT{c}", name=f"QT{c}")
              for c in range(NC_C)]
        Vt = [persist.tile([P, H, P], BF16, tag=f"V{i}", name=f"V{i}")
              for i in range(NT_FULL)]
        oT = [persist.tile([P, TOWN], BF16, tag=f"oT{c}", name=f"oT{c}")
              for c in range(NC_C)]
        ln2T = [persist.tile([P, TOWN], BF16, tag=f"l2T{c}", name=f"l2T{c}")
                for c in range(NC_C)]
        hT = [persist.tile([P, TOWN], BF16, tag=f"hT{m}", name=f"hT{m}")
              for m in range(NH_HID)]
        res = [persist.tile([P, C], F32, tag=f"res{j}", name=f"res{j}")
               for j in range(NT_OWN)]

        # ---- weights / constants ----
        qkvw_sb = [wpool.tile([P, 3 * C], BF16, tag=f"qkvw{c}", name=f"qkvw{c}")
                   for c in range(NC_C)]
        projw_sb = [wpool.tile([P, C], BF16, tag=f"projw{c}", name=f"projw{c}")
                    for c in range(NC_C)]
        fc1w_sb = [wpool.tile([P, HID], BF16, tag=f"fc1w{c}", name=f"fc1w{c}")
                   for c in range(NC_C)]
        fc2w_sb = [wpool.tile([P, C], BF16, tag=f"fc2w{m}", name=f"fc2w{m}")
                   for m in range(NH_HID)]

        ident = wpool.tile([P, P], BF16, tag="ident", name="ident")
        make_identity(nc, ident[:])
        eps_sb = wpool.tile([P, 1], F32, tag="eps", name="eps")
        nc.vector.memset(eps_sb[:], EPS)

        projb_bc = wpool.tile([P, C], F32, tag="projb", name="projb")
        fc2b_bc = wpool.tile([P, C], F32, tag="fc2b", name="fc2b")
        vb_bc = wpool.tile([P, C], F32, tag="vb", name="vb")
        nc.gpsimd.dma_start(out=vb_bc[:], in_=_bcast_ap(qkvb[ds(2 * C, C)]))
        qkvb_sb = wpool.tile([P, 9], F32, tag="qkvb", name="qkvb")
        nc.gpsimd.dma_start(qkvb_sb[:], qkvb.rearrange("(i p) -> p i", p=P))
        fc1b_sb = wpool.tile([P, NH_HID], F32, tag="fc1b", name="fc1b")
        nc.gpsimd.dma_start(fc1b_sb[:], fc1b.rearrange("(i p) -> p i", p=P))

        # ones columns of the augmented V (softmax denominator replication);
        # ones FIRST so the denominators land at PSUM partition base 0
        for i in range(NT_FULL):
            nc.vector.memset(Vt[i][:, :, 0:D], 1.0)

        def ln_apply(st_pool, xt, out_bf16):
            """(x - mean(x)) * rsqrt(var + eps); affine folded into weights."""
            stats = st_pool.tile([P, 6], F32, tag="stats", name="stats")
            nc.vector.bn_stats(out=stats[:], in_=xt[:])
            mv = st_pool.tile([P, 2], F32, tag="mv", name="mv")
            nc.vector.bn_aggr(out=mv[:], in_=stats[:])
            rstd = st_pool.tile([P, 1], F32, tag="rstd", name="rstd")
            nc.scalar.activation(out=rstd[:], in_=mv[:, 1:2], func=AF.Sqrt,
                                 bias=eps_sb[:], scale=1.0)
            nc.vector.reciprocal(out=rstd[:], in_=rstd[:])
            nc.vector.tensor_scalar(out=out_bf16[:], in0=xt[:], scalar1=mv[:, 0:1],
                                    scalar2=rstd[:], op0=OP.subtract, op1=OP.mult)


        def make_qkv_chunks(m, pool, tag="mm"):
            ks, qs = [], []
            for (o, w) in _chunks(TFULL):
                def gk(o=o, w=w, m=m, pool=pool, tag=tag):
                    ps = pool.tile([P, 512], F32, tag=tag, name="mm")
                    for c in range(NC_C):
                        nc.tensor.matmul(ps[:, :w],
                                         qkvw_sb[c][:, ds(C + m * P, P)],
                                         catT[c][:, ds(o, w)],
                                         start=(c == 0), stop=(c == NC_C - 1))
                    nc.vector.tensor_scalar_add(out=KT[m][:, ds(o, w)],
                                                in0=ps[:, :w],
                                                scalar1=qkvb_sb[:, 3 + m:4 + m])
                ks.append(gk)
            for (src_o, w, dst_o) in ((0, 512, 0), (512, 512, 512),
                                      (2048, P, 1024)):
                def gq(src_o=src_o, w=w, dst_o=dst_o, m=m, pool=pool, tag=tag):
                    ps = pool.tile([P, 512], F32, tag=tag, name="mm")
                    for c in range(NC_C):
                        nc.tensor.matmul(ps[:, :w],
                                         qkvw_sb[c][:, ds(m * P, P)],
                                         catT[c][:, ds(src_o, w)],
                                         start=(c == 0), stop=(c == NC_C - 1))
                    nc.vector.tensor_scalar_add(out=QT[m][:, ds(dst_o, w)],
                                                in0=ps[:, :w],
                                                scalar1=qkvb_sb[:, m:m + 1])
                qs.append(gq)
            return ks, qs

        def make_v(i, pool):
            def gv(i=i, pool=pool):
                ps = pool.tile([P, 512], F32, tag="mm", name="mm")
                for c in range(NC_C):
                    nc.tensor.matmul(ps[:, :C], catT[c][:, ts(i, P)],
                                     qkvw_sb[c][:, ds(2 * C, C)],
                                     start=(c == 0), stop=(c == NC_C - 1))
                nc.vector.tensor_add(Vt[i][:, :, D:P],
                                     ps[:, :C].rearrange("p (h d) -> p h d", h=H),
                                     vb_bc[:].rearrange("p (h d) -> p h d", h=H))
            return gv

        def make_proj(j, pool, tag="mm"):
            def gp(j=j, pool=pool, tag=tag):
                ps = pool.tile([P, 512], F32, tag=tag, name="mm")
                for c in range(NC_C):
                    nc.tensor.matmul(ps[:, :C], oT[c][:, ts(j, P)],
                                     projw_sb[c][:],
                                     start=(c == 0), stop=(c == NC_C - 1))
                nc.vector.tensor_add(res[j][:], ps[:, :C], res[j][:])
            return gp

        # ================= Phase A: LN1 + transpose to feature-major ========
        with ExitStack() as sa:
            xt_pool = sa.enter_context(tc.tile_pool(name="xt", bufs=3))
            st_pool = sa.enter_context(tc.tile_pool(name="lnstat", bufs=3))
            lt_pool = sa.enter_context(tc.tile_pool(name="lntok", bufs=3))
            tr_psum = sa.enter_context(tc.tile_pool(name="trps", bufs=3, space="PSUM"))
            mm_psum = sa.enter_context(tc.tile_pool(name="mmps", bufs=3, space="PSUM"))

            for c in range(NC_C):
                nc.sync.dma_start(qkvw_sb[c][:], qkvwT[ts(c, P), :])
            k0, q0 = make_qkv_chunks(0, mm_psum)
            after_tile = {3: [k0[0], q0[0]], 7: [k0[1], q0[1]], 11: [k0[2]],
                          15: [k0[3]], 16: [q0[2]], 17: [k0[4]]}
            for i in range(NT_FULL):
                xt = xt_pool.tile([P, C], F32, tag="xt", name="xt")
                nc.sync.dma_start(xt[:], tok_rows(i))
                if i in own_tiles:
                    nc.vector.tensor_add(res[own_tiles.index(i)][:], xt[:],
                                         projb_bc[:])
                ln_tok = lt_pool.tile([P, C], BF16, tag="lntok", name="lntok")
                ln_apply(st_pool, xt, ln_tok)
                for c in range(NC_C):
                    tp = tr_psum.tile([P, P], BF16, tag="tr", name="tr")
                    nc.tensor.transpose(tp[:], ln_tok[:, ts(c, P)], ident[:])
                    nc.scalar.copy(catT[c][:, ts(i, P)], tp[:])
                make_v(i, mm_psum)()
                for g in after_tile.pop(i, []):
                    g()
                if i == 3:
                    for c in range(NC_C):
                        nc.sync.dma_start(projw_sb[c][:], projwT[ts(c, P), :])
                        nc.sync.dma_start(fc1w_sb[c][:], fc1wT[ts(c, P), :])
                    for mm in range(NH_HID):
                        nc.sync.dma_start(fc2w_sb[mm][:], fc2wT[ts(mm, P), :])
                    nc.gpsimd.dma_start(out=projb_bc[:], in_=_bcast_ap(projb))
                    nc.gpsimd.dma_start(out=fc2b_bc[:], in_=_bcast_ap(fc2b))

        # ============ Phase B+C: QKV + attention, interleaved ===============
        # V and QKV(m=0) first; then attention(m) runs with QKV(m+1) matmul
        # groups sprinkled into its key-tile loop as TensorE filler while the
        # Scalar engine (exp) is the binding resource.
        with ExitStack() as sc:
            s_psum = sc.enter_context(tc.tile_pool(name="sps", bufs=3, space="PSUM"))
            o_psum = sc.enter_context(tc.tile_pool(name="ops", bufs=2, space="PSUM"))
            p_pool = sc.enter_context(tc.tile_pool(name="psb", bufs=5))
            r_pool = sc.enter_context(tc.tile_pool(name="recip", bufs=4))

            def norm_out(ops_t, m, po, qo, qw):
                rec = r_pool.tile([D, 512], F32, tag="rec", name="rec")
                nc.vector.reciprocal_approx_fast(out=rec[:, :qw],
                                                 in_=ops_t[0:D, :qw])
                nc.vector.tensor_mul(oT[m][po:po + D, ds(qo, qw)],
                                     ops_t[D:P, :qw], rec[:, :qw])

            for m in range(NC_C):
                if m + 1 < NC_C:
                    ks, qs = make_qkv_chunks(m + 1, s_psum, tag="s")
                    filler_ci = {0: ks[:2] + qs[:1], 1: ks[2:4] + qs[1:2]}
                    filler_y = ks[4:] + qs[2:]
                else:
                    filler_ci = {0: [],
                                 1: [make_proj(j, s_psum, tag="s") for j in range(4)]}
                    filler_y = [make_proj(j, s_psum, tag="s") for j in range(4, 8)]
                # ---- x-attention: 2 chunks of 512 queries ----
                for ci in range(2):
                    qo = ci * 512
                    opair = [o_psum.tile([P, 512], F32, tag="o", name="o")
                             for _ in range(2)]
                    for kt in range(NKT_X):
                        sps = s_psum.tile([P, 1024], F32, tag="s", name="s")
                        for par in range(2):
                            po = par * D
                            nc.tensor.matmul(sps[:, ds(par * 512, 512)],
                                             KT[m][po:po + D, ts(kt, P)],
                                             QT[m][po:po + D, ds(qo, 512)],
                                             tile_position=(po, 0))
                        pt = p_pool.tile([P, 1024], BF16, tag="p", name="p")
                        nc.scalar.activation(out=pt[:], in_=sps[:],
                                             func=AF.Exp, scale=SCALE)
                        for par in range(2):
                            nc.tensor.matmul(opair[par][:],
                                             Vt[kt][:, 2 * m + par, :],
                                             pt[:, ds(par * 512, 512)],
                                             start=(kt == 0), stop=(kt == NKT_X - 1))
                        if kt % 4 == 1 and filler_ci[ci]:
                            filler_ci[ci].pop(0)()
                    for par in range(2):
                        norm_out(opair[par], m, par * D, qo, 512)

                # ---- y-attention: 128 queries, 4 key tiles per S tile ----
                oy = [o_psum.tile([P, 512], F32, tag="o", name="o")
                      for _ in range(2)]
                groups = [list(range(g, min(g + 4, NKT_ALL)))
                          for g in range(0, NKT_ALL, 4)]
                for gkts in groups:
                    sps = s_psum.tile([P, 1024], F32, tag="s", name="s")
                    for gi, kt in enumerate(gkts):
                        for par in range(2):
                            po = par * D
                            nc.tensor.matmul(
                                sps[:, ds(par * 512 + gi * TOWN_Y, TOWN_Y)],
                                KT[m][po:po + D, ts(kt, P)],
                                QT[m][po:po + D, ds(TOWN_X, TOWN_Y)],
                                tile_position=(po, 0))
                    pt = p_pool.tile([P, 1024], BF16, tag="p", name="p")
                    nc.scalar.activation(out=pt[:], in_=sps[:],
                                         func=AF.Exp, scale=SCALE)
                    for gi, kt in enumerate(gkts):
                        for par in range(2):
                            nc.tensor.matmul(
                                oy[par][:, 0:TOWN_Y],
                                Vt[kt][:, 2 * m + par, :],
                                pt[:, ds(par * 512 + gi * TOWN_Y, TOWN_Y)],
                                start=(kt == 0), stop=(kt == NKT_ALL - 1))
                    if filler_y:
                        filler_y.pop(0)()
                for par in range(2):
                    norm_out(oy[par], m, par * D, TOWN_X, TOWN_Y)
                while filler_ci[0] or filler_ci[1] or filler_y:
                    (filler_ci[0] or filler_ci[1] or filler_y).pop(0)()
            make_proj(8, s_psum, tag="s")()

        # ============ Phase D/E/F: proj+residual, LN2, MLP ==================
        with ExitStack() as sd:
            mm_psum = sd.enter_context(tc.tile_pool(name="mmps2", bufs=2, space="PSUM"))
            tr_psum = sd.enter_context(tc.tile_pool(name="trps2", bufs=2, space="PSUM"))
            st_pool = sd.enter_context(tc.tile_pool(name="lnstat2", bufs=3))
            lt_pool = sd.enter_context(tc.tile_pool(name="lntok2", bufs=3))
            tmp_pool = sd.enter_context(tc.tile_pool(name="tmp", bufs=3))
            out_pool = sd.enter_context(tc.tile_pool(name="outp", bufs=3))

            for j in range(NT_OWN):
                ln_tok = lt_pool.tile([P, C], BF16, tag="lntok", name="lntok")
                ln_apply(st_pool, res[j], ln_tok)
                for c in range(NC_C):
                    tp = tr_psum.tile([P, P], BF16, tag="tr", name="tr")
                    nc.tensor.transpose(tp[:], ln_tok[:, ts(c, P)], ident[:])
                    nc.vector.tensor_copy(ln2T[c][:, ts(j, P)], tp[:])

            for m in range(NH_HID):
                for (o, w) in ((0, 1024), (1024, TOWN_Y)):
                    ps = mm_psum.tile([P, 1024], F32, tag="mmw", name="mmw", bufs=2)
                    for (f, fw) in _chunks(w):
                        for c in range(NC_C):
                            nc.tensor.matmul(ps[:, ds(f, fw)],
                                             fc1w_sb[c][:, ts(m, P)],
                                             ln2T[c][:, ds(o + f, fw)],
                                             start=(c == 0), stop=(c == NC_C - 1))
                    nc.scalar.activation(out=hT[m][:, ds(o, w)], in_=ps[:, :w],
                                         func=AF.Gelu, bias=fc1b_sb[:, m:m + 1],
                                         scale=1.0)

            for j in range(NT_OWN):
                ps = mm_psum.tile([P, 512], F32, tag="mm", name="mm")
                for m in range(NH_HID):
                    nc.tensor.matmul(ps[:, :C], hT[m][:, ts(j, P)], fc2w_sb[m][:],
                                     start=(m == 0), stop=(m == NH_HID - 1))
                t = tmp_pool.tile([P, C], F32, tag="mlp", name="mlp")
                nc.vector.tensor_add(t[:], ps[:, :C], fc2b_bc[:])
                ot = out_pool.tile([P, C], F32, tag="out", name="out")
                nc.vector.tensor_add(ot[:], t[:], res[j][:])
                if j < 8:
                    nc.sync.dma_start(out_x[ts(j, P), :], ot[:])
                else:
                    nc.sync.dma_start(out_y[:, :], ot[:])

            if debug_dump:
                for c in range(NC_C):
                    for nm, t in (("KT", KT), ("QT", QT), ("oT", oT),
                                  ("catT", catT)):
                        dt = nc.dram_tensor(f"d_{nm}{c}", list(t[c].shape), BF16,
                                            kind="ExternalOutput").ap()
                        nc.sync.dma_start(dt[:], t[c][:])
                for i in (0, 16):
                    dt = nc.dram_tensor(f"d_V{i}", list(Vt[i].shape), BF16,
                                        kind="ExternalOutput").ap()
                    nc.sync.dma_start(dt[:], Vt[i][:])

    nc.compile()
    return nc


_NC_CACHE = None


def _get_nc():
    global _NC_CACHE
    if _NC_CACHE is None:
        _NC_CACHE = build_nc()
    return _NC_CACHE


def make_in_maps(x, y, norm1_w, norm1_b, norm2_w, norm2_b, qkv_w,
                 proj_w, proj_b, fc1_w, fc1_b, fc2_w, fc2_b):
    bf = ml_dtypes.bfloat16
    f32 = np.float32
    qkv_w = np.asarray(qkv_w, f32)
    fc1_w = np.asarray(fc1_w, f32)
    n1w = np.asarray(norm1_w, f32)
    n1b = np.asarray(norm1_b, f32)
    n2w = np.asarray(norm2_w, f32)
    n2b = np.asarray(norm2_b, f32)
    qkv_w_eff = qkv_w * n1w[None, :]
    qkv_b_eff = qkv_w @ n1b
    fc1_w_eff = fc1_w * n2w[None, :]
    fc1_b_eff = np.asarray(fc1_b, f32) + fc1_w @ n2b
    shared = {
        "qkvwT": np.ascontiguousarray(qkv_w_eff.T.astype(bf)),
        "qkvb": np.ascontiguousarray(qkv_b_eff, f32),
        "projwT": np.ascontiguousarray(np.asarray(proj_w, f32).T.astype(bf)),
        "projb": np.ascontiguousarray(proj_b, f32),
        "fc1wT": np.ascontiguousarray(fc1_w_eff.T.astype(bf)),
        "fc1b": np.ascontiguousarray(fc1_b_eff, f32),
        "fc2wT": np.ascontiguousarray(np.asarray(fc2_w, f32).T.astype(bf)),
        "fc2b": np.ascontiguousarray(fc2_b, f32),
    }
    x = np.asarray(x, f32)
    y = np.asarray(y, f32)
    in_maps = []
    for core in range(N_CORES):
        b, s = core // 2, core % 2
        if s == 0:
            xbp, ybp = x[b], y[b]
        else:
            xbp = np.concatenate([x[b, TOWN_X:], x[b, :TOWN_X]], axis=0)
            ybp = np.concatenate([y[b, TOWN_Y:], y[b, :TOWN_Y]], axis=0)
        in_maps.append({"xb": np.ascontiguousarray(xbp),
                        "yb": np.ascontiguousarray(ybp), **shared})
    return in_maps


def assemble(results):
    new_x = np.empty((B, NX, C), np.float32)
    new_y = np.empty((B, NY, C), np.float32)
    for core in range(N_CORES):
        b, s = core // 2, core % 2
        new_x[b, s * TOWN_X:(s + 1) * TOWN_X] = results[core]["out_x"]
        new_y[b, s * TOWN_Y:(s + 1) * TOWN_Y] = results[core]["out_y"]
    return new_x, new_y


def run(in_maps, trace=False):
    return run_bass_kernel_spmd(_get_nc(), in_maps,
                                core_ids=list(range(N_CORES)), trace=trace)


def kernel(**inputs):
    in_maps = make_in_maps(**inputs)
    res = run(in_maps, trace=False)
    return assemble(res.results)
